# revision 5
# baseline (speedup 1.0000x reference)
"""GarNet layer kernel for Trainium2 (8 NeuronCores, data-parallel over batch).

Math (per example b):
    w    = exp(-d_av^2)                      [V=128, S=16]
    hi   = w^T @ fi_v / V                    [S, N=64]
    out  = mean_V(w)[:, None] * hi           [S, N] -> flattened [S*N]


Queue assignment (16 chunks x 32 examples, bpc=512):
  SP:   fi loads, even chunks            ~50.5us
  Pool: fi loads, odd chunks             ~50.5us
  ACT:  d loads + exp + stores           ~50.7us
  DVE:  memsets + squares + epilogue     ~38us
  PE:   512 matmuls (64 free x 4cyc) + 64 wbar matmuls  ~55us

wbar trick: per 8-example psum group, one extra matmul with
lhsT = the group's packed w block [128, 128] and rhs = a persistent
[128,1] ones/V^2 column writes col 64 of the bank: exactly
sum_v w[v,(e,s)]/V^2 at partition (e,s) -- the per-partition scalar
the epilogue needs.
"""

import numpy as np
from contextlib import ExitStack

import concourse.bass as bass
import concourse.tile as tile
from concourse import mybir

B, V, S, N = 4096, 128, 16, 64
NCORES = 8
BPC = B // NCORES
ONES_VAL = 1.0 / (V * V)


def split_multi_waits(nc):
    fn = nc.m.functions[0]
    for block in fn.blocks:
        insts = list(block.instructions)
        changed = False
        new = []
        for inst in insts:
            si = inst.sync_info
            waits = list(si.on_wait) if (si and si.on_wait) else []
            if len(waits) > 1:
                changed = True
                for w in waits:
                    ev = mybir.InstEventSemaphore(
                        name=nc.get_next_instruction_name(), ins=[], outs=[]
                    )
                    ev.engine = inst.engine
                    ev.sync_info = mybir.SyncInfo(on_wait=[w], on_update=[])
                    new.append(ev)
                ups = list(si.on_update) if si.on_update else []
                inst.sync_info = mybir.SyncInfo(on_wait=[], on_update=ups)
            new.append(inst)
        if changed:
            block.instructions = new


def build(bpc=BPC, e_chunk=32, name="garnet", split_waits=True):
    assert bpc % e_chunk == 0 and e_chunk % 8 == 0
    nchunk = bpc // e_chunk
    G = e_chunk // 8
    Q = e_chunk // 2

    nc = bass.Bass(name=name)
    fi = nc.dram_tensor("fi_v", (bpc, V, N), mybir.dt.float32, kind="ExternalInput")
    dav = nc.dram_tensor("d_av", (bpc, V, S), mybir.dt.float32, kind="ExternalInput")
    out = nc.dram_tensor("out", (bpc, S * N), mybir.dt.float32, kind="ExternalOutput")

    f32 = mybir.dt.float32
    with tile.TileContext(nc) as tc, ExitStack() as ctx:
        warmpool = ctx.enter_context(tc.tile_pool(name="warmpool", bufs=1))
        fipool = ctx.enter_context(tc.tile_pool(name="fipool", bufs=4))
        dpool = ctx.enter_context(tc.tile_pool(name="dpool", bufs=3))
        opool = ctx.enter_context(tc.tile_pool(name="opool", bufs=3))
        psum = ctx.enter_context(tc.tile_pool(name="psum", bufs=7, space="PSUM"))
        wpsum = ctx.enter_context(tc.tile_pool(name="wpsum", bufs=1, space="PSUM"))

        with tc.high_priority():
            wz = warmpool.tile([128, 1], f32)
            nc.vector.memset(wz, 0.0)
            wbig = warmpool.tile([128, 1], f32)
            nc.vector.memset(wbig, -88.0)
            wz2 = warmpool.tile([128, 128], f32)
            nc.vector.memset(wz2, 0.0)
            wps = wpsum.tile([128, 128], f32)
            # dummy matmul chain: keeps the PE continuously busy from t~0.5us
            # so the p-state ramp is mature when real matmuls arrive
            for _ in range(8):
                nc.tensor.matmul(
                    out=wps[0:1, :], lhsT=wz, rhs=wz2, start=True, stop=True
                )
        wzero = warmpool.tile([128, 1], f32)

        # two 16-example warmup chunks, then steady-state 32s
        sizes = [16, 16] + [24] * ((bpc - 32) // 24)
        assert sum(sizes) == bpc
        pending_store = None
        b0 = 0
        for c, E in enumerate(sizes):
            G, Q = E // 8, E // 2
            fi_t = fipool.tile([128, E, N + 1], f32)
            nc.vector.memset(fi_t[:, :, N : N + 1], ONES_VAL)
            if c == 0:
                # warmup: split the first load across both queues
                for k in range(2):
                    eng = nc.sync if k == 0 else nc.gpsimd
                    eng.dma_start(
                        out=fi_t[:, 8 * k : 8 * k + 8, 0:N],
                        in_=fi[b0 + 8 * k : b0 + 8 * k + 8].rearrange(
                            "e v n -> v e n"
                        ),
                    )
            else:
                fi_eng = nc.sync if c % 2 == 0 else nc.gpsimd
                fi_eng.dma_start(
                    out=fi_t[:, :, 0:N],
                    in_=fi[b0 : b0 + E].rearrange("e v n -> v e n"),
                )
            # d chunk -> [V, pair, slot, S], slots (w_even, 0, w_odd)
            d_t = dpool.tile([128, Q, 3, S], f32)
            nc.vector.memset(d_t[:, :, 1, :], 0.0)
            dsrc = dav[b0 : b0 + E].rearrange("(q t) v s -> t v q s", t=2)
            for t in range(2):
                nc.scalar.dma_start(out=d_t[:, :, 2 * t, :], in_=dsrc[t])
            wslots = d_t[:, :, 0:3:2, :]
            if c == 0:
                # warm op: loads the act table while chunk 0's DMAs land and
                # produces the exact-zero bias column used by chunk 0's exp
                # (the data dependency keeps the scheduler from deferring it)
                nc.scalar.activation(
                    wzero, wz, mybir.ActivationFunctionType.Exp,
                    scale=0.0, bias=wbig,
                )
                nc.scalar.activation(
                    wslots, wslots, mybir.ActivationFunctionType.Square
                )
                nc.scalar.activation(
                    wslots, wslots, mybir.ActivationFunctionType.Exp,
                    scale=-1.0, bias=wzero,
                )
            else:
                nc.vector.tensor_mul(wslots, wslots, wslots)
                nc.scalar.activation(
                    wslots, wslots, mybir.ActivationFunctionType.Exp, scale=-1.0
                )

            # previous chunk's store, after this chunk's loads are queued
            if pending_store is not None:
                st_dst, st_src = pending_store
                nc.scalar.dma_start(out=st_dst, in_=st_src)

            o_t = opool.tile([128, G, N], f32)
            for g in range(G):
                ps = psum.tile([128, N + 1], f32)
                for jj in range(8):
                    e = g * 8 + jj
                    q, t = e // 2, e % 2
                    nc.tensor.matmul(
                        out=ps[32 * (jj // 2) : 32 * (jj // 2) + 32, :],
                        lhsT=d_t[:, q, t : t + 2, :],
                        rhs=fi_t[:, e, :],
                        start=(t == 0),
                        stop=(t == 1),
                        tile_position=(0, 32 * (jj // 2)),
                    )
                nc.vector.tensor_scalar_mul(o_t[:, g, :], ps[:, 0:N], ps[:, N : N + 1])

            if c < len(sizes) - 1:
                dst = out[b0 : b0 + E].rearrange(
                    "(g jj) (s n) -> (jj s) g n", jj=8, s=S
                )
                pending_store = (dst, o_t)
            else:
                # drain fast: store the last chunk per group
                for g in range(G):
                    dst_g = out[b0 + 8 * g : b0 + 8 * g + 8].rearrange(
                        "jj (s n) -> (jj s) n", s=S
                    )
                    nc.scalar.dma_start(out=dst_g, in_=o_t[:, g, :])
            b0 += E

    if split_waits:
        split_multi_waits(nc)
    return nc




_NC_CACHE = {}


def _get_nc():
    if "nc" not in _NC_CACHE:
        _NC_CACHE["nc"] = build()
    return _NC_CACHE["nc"]


def kernel(fi_v: np.ndarray, d_av: np.ndarray) -> np.ndarray:
    from concourse.bass_utils import run_bass_kernel_spmd

    fi_v = np.ascontiguousarray(np.asarray(fi_v, dtype=np.float32))
    d_av = np.ascontiguousarray(np.asarray(d_av, dtype=np.float32))
    assert fi_v.shape == (B, V, N) and d_av.shape == (B, V, S)
    nc = _get_nc()
    in_maps = [
        {
            "fi_v": fi_v[c * BPC : (c + 1) * BPC],
            "d_av": d_av[c * BPC : (c + 1) * BPC],
        }
        for c in range(NCORES)
    ]
    res = run_bass_kernel_spmd(nc, in_maps, core_ids=list(range(NCORES)))
    return np.concatenate([res.results[c]["out"] for c in range(NCORES)], axis=0)

